# revision 22
# baseline (speedup 1.0000x reference)
"""DTNNStep Bass kernel for Trainium2 (8 NeuronCores, data-parallel over batch).

Computes, per molecule b:
    dist_h = dist @ W_df + b_df              # [N, N, H]
    atom_h = atom @ W_cf + b_cf              # [N, H]
    gated  = dist_h * atom_h[None, :, :]     # broadcast over i
    out    = tanh((gated @ W_fc) * mask)     # mask == 1 in this benchmark
    result = out.sum(axis=1) + atom          # [N, F]

v2 dataflow (per-j tiles from a natural-layout load):
  - dist is DMA'd in its natural [i, (j d)] layout (3.2KB contiguous
    descriptors, f32->bf16 cast on the SWDGE wire) instead of the old
    small-packet "i j d -> j i d" rearrange.
  - Per j, the [i, d] slice is PE-transposed to [d, i]; 8 j's batch into one
    PSUM bank, evacuated to SBUF (alternating Scalar/Vector) into a persistent
    buffer whose partition row 100 holds ones (b_df bias fold for mm1).
  - mm1 runs as a col-tiled concurrent pair (partitions 0:64 / 64:128 of the
    output hold two 4-j groups), mm2 as a diag-tiled concurrent pair.
  - The j-reduction of tanh runs on the PE: each [f-pack, i] tanh slice is
    transpose-ACCUMULATED (start/stop flags) into a per-molecule PSUM bank,
    which ends up holding sum_j tanh in [i, f|f] orientation.  No DVE
    tensor_reduce, no final transpose.
"""

import os
import sys

import numpy as np

for _p in ("/opt/trn_rl_repo", os.path.expanduser("~/.axon_site/_ro/trn_rl_repo")):
    if os.path.isdir(_p) and _p not in sys.path:
        sys.path.insert(0, _p)

import concourse.bass as bass
import concourse.tile as tile
from concourse import bacc, mybir
from concourse.bass import ds
from concourse.bass_utils import run_bass_kernel_spmd
from concourse.masks import make_identity

B, N, NF, ND, NH = 16, 128, 64, 100, 64
NCORES = 8
BPC = B // NCORES  # molecules per core

F32 = mybir.dt.float32
BF16 = mybir.dt.bfloat16

JG = 8  # j's per unit (one PSUM transpose bank)
NU = N // JG  # units per molecule
DT_SLOTS = 4  # rotating distT buffers

# Pad transpose stationary loads to 128 columns (FWL eligibility bet).
PAD_TRANSPOSE = False


def _emit(tc):
    nc = tc.nc
    dist = nc.dram_tensor("dist", (BPC, N, N, ND), F32, kind="ExternalInput").ap()
    atom = nc.dram_tensor("atom", (BPC, N, NF), F32, kind="ExternalInput").ap()
    w_cf = nc.dram_tensor("w_cf", (NF, NH), F32, kind="ExternalInput").ap()
    w_df = nc.dram_tensor("w_df", (ND, NH), F32, kind="ExternalInput").ap()
    w_fc = nc.dram_tensor("w_fc", (NH, NF), F32, kind="ExternalInput").ap()
    b_cf = nc.dram_tensor("b_cf", (1, NH), F32, kind="ExternalInput").ap()
    b_df = nc.dram_tensor("b_df", (1, NH), F32, kind="ExternalInput").ap()
    out = nc.dram_tensor("out", (BPC, N, NF), F32, kind="ExternalOutput").ap()

    with (
        tc.tile_pool(name="consts", bufs=1) as consts,
        tc.tile_pool(name="loads", bufs=8) as loads,
        tc.tile_pool(name="small", bufs=2) as small,
        tc.tile_pool(name="work", bufs=3) as work,
        tc.tile_pool(name="ppool", bufs=2, space="PSUM") as ppool,
    ):
        atom_ins = []
        for b in range(BPC):
            atom_in = small.tile([N, NF], F32, tag="atom_in")
            nc.sync.dma_start(atom_in, atom[b])
            atom_ins.append(atom_in)

        identity = consts.tile([128, 128], F32)
        make_identity(nc, identity)
        identity_bf = consts.tile([128, 128], BF16)
        make_identity(nc, identity_bf)

        ones_bf = consts.tile([1, N], BF16)
        nc.vector.memset(ones_bf, 1.0)

        # Preload the tanh table set while the first DMAs are in flight.
        warm_src = consts.tile([1, 8], F32)
        nc.vector.memset(warm_src, 0.5)
        warm_tanh = consts.tile([1, 8], F32)
        nc.scalar.activation(warm_tanh, warm_src, mybir.ActivationFunctionType.Tanh)

        # W_df augmented with b_df as row ND; mm1 consumes distT with a ones
        # row at partition ND so the bias folds in for free.
        w_df_aug_f = consts.tile([ND + 1, NH], F32)
        nc.sync.dma_start(w_df_aug_f[:ND], w_df)
        nc.sync.dma_start(w_df_aug_f[ND : ND + 1], b_df)
        w_df_aug = consts.tile([ND + 1, NH], BF16)
        nc.vector.tensor_copy(w_df_aug, w_df_aug_f)

        # W_fc stacked twice vertically for the diag-tiled mm2 pair.
        w_fc_f = consts.tile([2 * NH, NF], F32)
        nc.sync.dma_start(w_fc_f[:NH], w_fc)
        nc.sync.dma_start(w_fc_f[NH:], w_fc)
        w_fc2 = consts.tile([2 * NH, NF], BF16)
        nc.vector.tensor_copy(w_fc2, w_fc_f)

        # atom_h path constants (bf16 so the shifted col-tiled pair is legal).
        w_cf_f = consts.tile([NF, NH], F32)
        nc.sync.dma_start(w_cf_f, w_cf)
        w_cf_bf = consts.tile([NF, NH], BF16)
        nc.vector.tensor_copy(w_cf_bf, w_cf_f)
        b_cf_f = consts.tile([1, NH], F32)
        nc.sync.dma_start(b_cf_f, b_cf)
        b_cf_bf = consts.tile([1, NH], BF16)
        nc.vector.tensor_copy(b_cf_bf, b_cf_f)

        # Persistent transposed-dist buffers; partition row ND stays 1.0
        # (everything below it is overwritten by each unit's evacuation).
        distT_buf = consts.tile([ND + 1, DT_SLOTS * JG * N], BF16)
        nc.vector.memset(distT_buf, 1.0)

        # --- per-molecule prologue: ah2[p, j] where rows 0:64 hold
        # atom_h^T[h, j] and rows 64:128 hold atom_h^T[h, j+4] (so one gate AP
        # serves both partition halves of the packed unit).
        ah2s = []
        res_tiles = []
        for b in range(BPC):
            atom_in = atom_ins[b]
            res_ps = ppool.tile([128, 512], F32, tag="res")
            res_tiles.append(res_ps)

            atomT_ps = res_ps[:NF, ds(128, N)]
            nc.tensor.transpose(atomT_ps, atom_in, identity)
            atomT_bf = small.tile([NF, N], BF16, tag="atomT")
            nc.vector.tensor_copy(atomT_bf, atomT_ps)

            ah_ps = res_ps[:, ds(256, N)]
            nc.tensor.matmul(ah_ps[:NH], w_cf_bf, atomT_bf, start=True, stop=False)
            nc.tensor.matmul(
                ah_ps[:NH], b_cf_bf, ones_bf, start=False, stop=True
            )
            nc.tensor.matmul(
                ah_ps[NH:, : N - 4], w_cf_bf, atomT_bf[:, 4:], start=True, stop=False
            )
            nc.tensor.matmul(
                ah_ps[NH:, : N - 4], b_cf_bf, ones_bf[:, : N - 4],
                start=False, stop=True,
            )
            ah2 = small.tile([2 * NH, N], BF16, tag="ah2")
            nc.vector.tensor_copy(ah2[:, : N - 4], ah_ps[:, : N - 4])
            nc.vector.tensor_copy(ah2[:NH, N - 4 :], ah_ps[:NH, N - 4 :])
            ah2s.append(ah2)

        units = [(b, jg) for b in range(BPC) for jg in range(NU)]
        NUNITS = len(units)
        st = {}  # per-unit pipeline state
        def stage_a(u):
            b, jg = units[u]
            # load chunk: dist[b, :, 8jg:8jg+8, :] -> [i, (j d)] bf16
            chunk = loads.tile([N, JG * ND], BF16, tag="chunk")
            nc.gpsimd.dma_start(
                chunk,
                dist[b].rearrange("i j d -> i (j d)")[:, ds(jg * JG * ND, JG * ND)],
            )
            # 8 PE transposes [i, d] -> [d, i] into one PSUM bank
            tp = ppool.tile([128, JG * N], BF16, tag="tp")
            for q in range(JG):
                if PAD_TRANSPOSE and q < JG - 1:
                    nc.tensor.transpose(
                        tp[:, ds(q * N, N)], chunk[:, ds(q * ND, 128)], identity_bf
                    )
                else:
                    nc.tensor.transpose(
                        tp[:ND, ds(q * N, N)], chunk[:, ds(q * ND, ND)], identity_bf
                    )
            # evacuate to SBUF (alternate engines); ones row persists
            slot = u % DT_SLOTS
            distT = distT_buf[:, ds(slot * JG * N, JG * N)]
            if u % 3 == 0:
                nc.scalar.copy(distT[:ND], tp[:ND])
            else:
                nc.vector.tensor_copy(distT[:ND], tp[:ND])
            st[u] = {"distT": distT}

        def stage_m1(u):
            b, jg = units[u]
            distT = st[u]["distT"]
            # mm1 col-tiled pair: out1[0:64] <- j's 0:4, [64:128] <- 4:8
            out1 = ppool.tile([128, 4 * N], F32, tag="out1")
            nc.tensor.matmul(
                out1[:NH], w_df_aug, distT[: ND + 1, : 4 * N], start=True, stop=True
            )
            nc.tensor.matmul(
                out1[NH:], w_df_aug, distT[: ND + 1, 4 * N :], start=True, stop=True
            )
            # gate: gated[p, (jq, i)] = out1 * ah[p-half, 8jg+jq(+4)]
            ah2 = ah2s[b]
            gated = work.tile([128, 4 * N], BF16, tag="gated", bufs=4)
            nc.vector.tensor_tensor(
                gated.rearrange("p (jq i) -> p jq i", jq=4),
                out1.rearrange("p (jq i) -> p jq i", jq=4),
                ah2[:, ds(jg * JG, 4), None].to_broadcast((2 * NH, 4, N)),
                mybir.AluOpType.mult,
            )
            st[u]["gated"] = gated

        def stage_b(u):
            gated = st[u]["gated"]
            out2 = ppool.tile([128, 4 * N], F32, tag="out2")
            nc.tensor.matmul(out2[:NF], w_fc2[:NH], gated[:NH], start=True, stop=True)
            nc.tensor.matmul(out2[NF:], w_fc2[NH:], gated[NH:], start=True, stop=True)
            tanh_sb = work.tile([128, 4 * N], BF16, tag="tanh_sb", bufs=4)
            nc.scalar.activation(tanh_sb, out2, mybir.ActivationFunctionType.Tanh)
            st[u]["tanh_sb"] = tanh_sb

        def stage_c(u):
            b, jg = units[u]
            # accumulate the whole [128, (jq, i)] tile into the molecule bank
            nc.tensor.matmul(
                res_tiles[b],
                identity_bf,
                st[u]["tanh_sb"],
                start=(jg == 0),
                stop=(jg == NU - 1),
                skip_group_check=True,
            )
            del st[u]

        def finalize(b):
            # res512[p, (jq, i)] = sum_units tanh; reduce jq on DVE, then
            # transpose halves and add atom.
            res_r = work.tile([128, N], F32, tag="res_r")
            nc.vector.tensor_reduce(
                res_r,
                res_tiles[b].rearrange("p (jq i) -> p i jq", jq=4),
                axis=mybir.AxisListType.X,
                op=mybir.AluOpType.add,
            )
            psA = ppool.tile([128, 4 * N], F32, tag="out1")
            nc.tensor.matmul(
                psA[:, :NF], res_r[:NF], identity[:NF, :NF],
                is_transpose=True, start=True, stop=True,
            )
            psB = ppool.tile([128, 4 * N], F32, tag="out2")
            nc.tensor.matmul(
                psB[:, :NF], res_r[NF:], identity[ds(NF, NF), ds(NF, NF)],
                is_transpose=True, start=True, stop=True,
            )
            tmp_sb = work.tile([N, NF], F32, tag="tmp_sb")
            nc.vector.tensor_add(tmp_sb, psA[:, :NF], atom_ins[b])
            out_sb = work.tile([N, NF], F32, tag="out_sb")
            nc.vector.tensor_add(out_sb, tmp_sb, psB[:, :NF])
            nc.sync.dma_start(out[b], out_sb)

        NPAIRS = NUNITS // 2
        for it in range(NPAIRS + 3):
            if it < NPAIRS:
                stage_a(2 * it)
                stage_a(2 * it + 1)
            if 1 <= it <= NPAIRS:
                stage_m1(2 * (it - 1))
                stage_m1(2 * (it - 1) + 1)
            if 2 <= it <= NPAIRS + 1:
                stage_b(2 * (it - 2))
                stage_b(2 * (it - 2) + 1)
            if it >= 3:
                stage_c(2 * (it - 3))
                stage_c(2 * (it - 3) + 1)
                b, jg = units[2 * (it - 3) + 1]
                if jg == NU - 1:
                    finalize(b)


_NC_CACHE = None


def _get_nc():
    global _NC_CACHE
    if _NC_CACHE is None:
        nc = bacc.Bacc("TRN2", target_bir_lowering=False, debug=False)
        with tile.TileContext(nc) as tc:
            _emit(tc)
        nc.compile()
        _NC_CACHE = nc
    return _NC_CACHE


def _numpy_reference(atom, dist, mask, w_cf, w_df, w_fc, b_cf, b_df):
    dist_h = np.einsum("bijd,dh->bijh", dist, w_df) + b_df
    atom_h = np.einsum("bjf,fh->bjh", atom, w_cf) + b_cf
    gated = dist_h * atom_h[:, None, :, :]
    o = np.einsum("bijh,hf->bijf", gated, w_fc)
    o = np.tanh(o * mask[..., None])
    return (o.sum(axis=2) + atom).astype(np.float32)


def run_sharded(inputs, trace=False):
    """Shard over the batch axis, run on 8 cores, gather. Returns (out, results)."""
    atom = np.ascontiguousarray(np.asarray(inputs["atom_features"], np.float32))
    dist = np.ascontiguousarray(np.asarray(inputs["distance_matrix"], np.float32))
    w_cf = np.ascontiguousarray(np.asarray(inputs["W_cf"], np.float32))
    w_df = np.ascontiguousarray(np.asarray(inputs["W_df"], np.float32))
    w_fc = np.ascontiguousarray(np.asarray(inputs["W_fc"], np.float32))
    b_cf = np.asarray(inputs["b_cf"], np.float32).reshape(1, NH)
    b_df = np.asarray(inputs["b_df"], np.float32).reshape(1, NH)

    nc = _get_nc()
    in_maps = []
    for c in range(NCORES):
        sl = slice(c * BPC, (c + 1) * BPC)
        in_maps.append(
            {
                "dist": dist[sl],
                "atom": atom[sl],
                "w_cf": w_cf,
                "w_df": w_df,
                "w_fc": w_fc,
                "b_cf": b_cf,
                "b_df": b_df,
            }
        )
    res = run_bass_kernel_spmd(nc, in_maps, core_ids=list(range(NCORES)), trace=trace)
    out = np.concatenate([res.results[c]["out"] for c in range(NCORES)], axis=0)
    return out, res


def kernel(**inputs) -> np.ndarray:
    mask = np.asarray(inputs["distance_matrix_mask"], np.float32)
    if not np.all(mask == 1.0):
        # The hardware pipeline folds the (always-ones) mask away; keep a
        # correct path for arbitrary masks.
        return _numpy_reference(
            np.asarray(inputs["atom_features"], np.float32),
            np.asarray(inputs["distance_matrix"], np.float32),
            mask,
            np.asarray(inputs["W_cf"], np.float32),
            np.asarray(inputs["W_df"], np.float32),
            np.asarray(inputs["W_fc"], np.float32),
            np.asarray(inputs["b_cf"], np.float32),
            np.asarray(inputs["b_df"], np.float32),
        )
    out, _ = run_sharded(inputs)
    return out
